# revision 37
# baseline (speedup 1.0000x reference)
"""CopyMechanism (pointer-generator) kernel for 8 Trainium2 NeuronCores.

Full problem: B=16, T=128, H=512, V=32000, S=400.
  gen = sigmoid(ctx@wh + hid@ws + trg@wx + b)          [B,T,1]
  out = gen * vocab_dists; out[b,t,ids[b,t,s]] += (1-gen)*attn[b,t,s]

Sharding: data-parallel over batch. Core i handles batches [2i, 2i+1]
(256 rows of T-steps). Weights replicated. No cross-core communication.

Device algorithm (per core, per row r):
  Decompose vocab index v = p*250 + f  (V = 128*250), so a row's 32000-wide
  output is an SBUF tile [128 partitions, 250 free].  The scatter-add of the
  S=400 attn values becomes a sum of outer products:
     M[p,f] = sum_s onehot(pi[s])[p] * (onehot(fi[s])[f] * val[s])
  computed by TensorE matmuls contracting s (4 chunks of <=128 on the
  partition axis).  One-hots are built on VectorE with iota/is_equal
  tensor_scalar ops in bf16 (indices pre-decomposed and pre-transposed on
  host -- integer-only preprocessing); A carries the scatter value.  The
  base p_gen*vocab is a 5th fp32 matmul with lhsT = p_gen*I (diagonal,
  built per row on ScalarE) that starts the PSUM accumulation group;
  ScalarE copies PSUM->SBUF and the store goes out on the ACT HWDGE ring
  (loads on the SP ring) so loads and stores don't serialize on one FIFO.

  p_gen is computed on-device (dot products + sigmoid), bounced through a
  DRAM scratch and re-loaded with a partition-broadcast AP so each row's
  scalar is available on all 128 partitions.
"""

import numpy as np
from ml_dtypes import bfloat16

# ---------------------------------------------------------------------------
# problem constants (hardcoded per contract)
B, T, H, V, S = 16, 128, 512, 32000, 400
N_CORES = 8
BPC = B // N_CORES          # batches per core
R_FULL = BPC * T            # rows per core = 256
FD_FULL = V // 128          # 250
SP_FULL = (S + 127) // 128  # 4 s-chunks
G_FULL = 16                 # rows per vocab DMA group

_PROGRAM_CACHE = {}


def build_program(R=R_FULL, FD=FD_FULL, SP=SP_FULL, G=G_FULL, mode="diag",
                  rep=1, a_engine="dve", ablate="full", pair_psum=True):
    """Build + compile the per-core Bass program. Same program for all cores.

    R : rows per core (multiple of 128)
    FD: free-dim width of the vocab decomposition (V_local = 128*FD)
    SP: number of 128-wide s-chunks (S padded to SP*128)
    G : rows per vocab/out DMA group
    mode: "diag" -> base p_gen*vocab via a diagonal matmul starting the PSUM
          group; "dve" -> base+merge on VectorE after the scatter matmuls.
    rep : repeat the whole body rep times (identical output; used for
          differential device-time measurement).
    """
    key = (R, FD, SP, G, mode, rep, a_engine, ablate, pair_psum)
    if key in _PROGRAM_CACHE:
        return _PROGRAM_CACHE[key]

    from contextlib import ExitStack

    import concourse.bass as bass
    import concourse.tile as tile
    from concourse import bacc, mybir

    f32 = mybir.dt.float32
    bf16 = mybir.dt.bfloat16
    Alu = mybir.AluOpType
    Act = mybir.ActivationFunctionType
    VL = 128 * FD
    RB = R // 128
    NG = R // G
    assert R % 128 == 0 and R % G == 0

    nc = bacc.Bacc("TRN2", target_bir_lowering=False, debug=False)

    ctx_d = nc.dram_tensor("ctx", [R, H], f32, kind="ExternalInput")
    hid_d = nc.dram_tensor("hid", [R, H], f32, kind="ExternalInput")
    trg_d = nc.dram_tensor("trg", [R, H], f32, kind="ExternalInput")
    vocab_d = nc.dram_tensor("vocab", [R, VL], f32, kind="ExternalInput")
    attnT_d = nc.dram_tensor("attnT", [128, RB * SP, 128], f32, kind="ExternalInput")
    piT_d = nc.dram_tensor("piT", [128, RB * SP, 128], f32, kind="ExternalInput")
    fiT_d = nc.dram_tensor("fiT", [128, RB * SP, 128], f32, kind="ExternalInput")
    # weights replicated across partitions on host (pure data movement)
    wh_d = nc.dram_tensor("wh", [128, H], f32, kind="ExternalInput")
    ws_d = nc.dram_tensor("ws", [128, H], f32, kind="ExternalInput")
    wx_d = nc.dram_tensor("wx", [128, H], f32, kind="ExternalInput")
    wxb_d = nc.dram_tensor("wxb", [128, 1], f32, kind="ExternalInput")
    iotaP_d = nc.dram_tensor("iotaP", [128, 128], bf16, kind="ExternalInput")
    iotaF_d = nc.dram_tensor("iotaF", [128, FD], bf16, kind="ExternalInput")
    ident_d = nc.dram_tensor("ident", [128, 128], f32, kind="ExternalInput")
    out_d = nc.dram_tensor("out", [R, VL], f32, kind="ExternalOutput")

    with tile.TileContext(nc) as tc, ExitStack() as es:
        singles = es.enter_context(tc.tile_pool(name="singles", bufs=1))
        ph1 = es.enter_context(tc.tile_pool(name="ph1", bufs=2))
        gbufs = 3 if G <= 16 else 2
        vpool = es.enter_context(tc.tile_pool(name="vpool", bufs=gbufs))
        opool = es.enter_context(tc.tile_pool(name="opool", bufs=gbufs))
        abpool = es.enter_context(tc.tile_pool(name="ab", bufs=6))
        ppool = es.enter_context(tc.tile_pool(name="psum", bufs=8, space="PSUM"))
        dpool = es.enter_context(tc.tile_pool(name="dram", bufs=1, space="DRAM"))

        # --- constants / small inputs ---
        attnT = singles.tile([128, RB * SP, 128], f32)
        nc.sync.dma_start(attnT[:], attnT_d[:])
        piT = singles.tile([128, RB * SP, 128], f32)
        nc.sync.dma_start(piT[:], piT_d[:])
        fiT = singles.tile([128, RB * SP, 128], f32)
        nc.sync.dma_start(fiT[:], fiT_d[:])
        iotaP = singles.tile([128, 128], bf16)
        nc.sync.dma_start(iotaP[:], iotaP_d[:])
        iotaF = singles.tile([128, FD], bf16)
        nc.sync.dma_start(iotaF[:], iotaF_d[:])
        ident = singles.tile([128, 128], f32)
        nc.sync.dma_start(ident[:], ident_d[:])
        wh = singles.tile([128, H], f32)
        nc.sync.dma_start(wh[:], wh_d[:])
        ws = singles.tile([128, H], f32)
        nc.sync.dma_start(ws[:], ws_d[:])
        wx = singles.tile([128, H], f32)
        nc.sync.dma_start(wx[:], wx_d[:])
        wxb = singles.tile([128, 1], f32)
        nc.sync.dma_start(wxb[:], wxb_d[:])
        scaledT = singles.tile([128, RB * SP, 128], f32)
        pgen_all = singles.tile([128, R], f32)
        om_all = singles.tile([128, R], f32)
        pgen_dram = dpool.tile([R, 1], f32)

        # --- phase 1a: p_gen per row (rows on partitions), bounce to DRAM ---
        def _phase1a():
          for blk in range(RB):
            rows = slice(blk * 128, (blk + 1) * 128)
            gacc = ph1.tile([128, 1], f32, tag="gacc")
            gtmp = ph1.tile([128, 1], f32, tag="gtmp")
            g2 = ph1.tile([128, 1], f32, tag="g2")
            prod = ph1.tile([128, H], f32, tag="prod")
            for i, (src_d, w) in enumerate(
                ((ctx_d, wh), (hid_d, ws), (trg_d, wx))
            ):
                x = ph1.tile([128, H], f32, tag="x")
                nc.sync.dma_start(x[:], src_d[rows, :])
                nc.vector.tensor_tensor(prod[:], x[:], w[:], op=Alu.mult)
                dst = (gacc, gtmp, g2)[i]
                nc.vector.tensor_reduce(
                    dst[:], prod[:], axis=mybir.AxisListType.X, op=Alu.add
                )
            gsum = ph1.tile([128, 1], f32, tag="gsum")
            nc.vector.tensor_tensor(gsum[:], gacc[:], gtmp[:], op=Alu.add)
            gall = ph1.tile([128, 1], f32, tag="gall")
            nc.vector.tensor_tensor(gall[:], gsum[:], g2[:], op=Alu.add)
            pgen_col = ph1.tile([128, 1], f32, tag="pgen")
            nc.scalar.activation(
                pgen_col[:], gall[:], Act.Sigmoid, bias=wxb[:], scale=1.0
            )
            nc.sync.dma_start(pgen_dram[rows, :], pgen_col[:])

        # --- phase 1b: broadcast p_gen to all partitions; scaled attnT ---
        def _phase1b():
            pg_flat = pgen_dram[:, 0]
            pg_bcast = bass.AP(
                tensor=pg_flat.tensor, offset=pg_flat.offset,
                ap=[[0, 128]] + list(pg_flat.ap),
            )
            nc.gpsimd.dma_start(pgen_all[:], pg_bcast)
            nc.vector.tensor_scalar(
                om_all[:], pgen_all[:], -1.0, 1.0, Alu.mult, Alu.add
            )
            for blk in range(RB):
                for c in range(SP):
                    nc.vector.tensor_tensor(
                        scaledT[:, blk * SP + c, :],
                        attnT[:, blk * SP + c, :],
                        om_all[:, blk * 128:(blk + 1) * 128],
                        op=Alu.mult,
                    )

        # --- phase 2: per-row scatter-add via one-hot matmuls ---
        vocab_v = vocab_d[:].rearrange("r (p f) -> p r f", p=128)
        out_v = out_d[:].rearrange("r (p f) -> p r f", p=128)

        def _phase2():
          for grp in range(NG):
            gr = slice(grp * G, (grp + 1) * G)
            ot = opool.tile([128, G, FD], f32)
            if mode == "dma":
                # Pre-fill ot with per-row p_gen, then the vocab load DMA
                # multiplies in transit: ot = p_gen * vocab (no PE/DVE time).
                for j in range(G):
                    r = grp * G + j
                    # ot[:, j, :] = 0*iotaF + p_gen[r]  (no broadcast APs)
                    nc.scalar.activation(
                        ot[:, j, :], iotaF[:], Act.Identity,
                        bias=pgen_all[:, r:r + 1], scale=0.0,
                    )
                nc.gpsimd.dma_start(
                    ot[:], vocab_v[:, gr, :], accum_op=Alu.mult
                )
            else:
                vt = vpool.tile([128, G, FD], f32)
                nc.sync.dma_start(vt[:], vocab_v[:, gr, :])
            if ablate == "dmaonly":
                if mode != "dma":
                    nc.scalar.copy(ot[:, :, :], vt[:, :, :])
                nc.scalar.dma_start(out_v[:, gr, :], ot[:])
                continue
            psb = None
            for j in range(G):
                r = grp * G + j
                blk = r // 128
                rl = r % 128
                if ablate == "nomm":
                    nc.scalar.copy(ot[:, j, :], vt[:, j, :])
                if pair_psum and mode == "diag":
                    if j % 2 == 0:
                        psb = ppool.tile([128, 2, 256], f32)
                    ps = psb[:, j % 2, 0:FD]
                else:
                    ps = ppool.tile([128, FD], f32)[:]
                pg_sc = pgen_all[:, r:r + 1]
                if ablate == "nomm":
                    for c in range(SP):
                        ch = blk * SP + c
                        A = abpool.tile([128, 128], bf16, tag="A")
                        eng = nc.gpsimd if a_engine == "gpsimd" else nc.vector
                        eng.tensor_scalar(
                            A[:], iotaP[:], piT[:, ch, rl:rl + 1],
                            scaledT[:, ch, rl:rl + 1], Alu.is_equal, Alu.mult,
                        )
                        Bt = abpool.tile([128, FD], bf16, tag="B")
                        nc.vector.tensor_scalar(
                            Bt[:], iotaF[:], fiT[:, ch, rl:rl + 1], None,
                            Alu.is_equal,
                        )
                    continue
                if mode == "diag":
                    D = abpool.tile([128, 128], f32, tag="D")
                    nc.scalar.mul(D[:], ident[:], pg_sc)
                    nc.tensor.matmul(
                        ps, lhsT=D[:], rhs=vt[:, j, :],
                        start=(j % 2 == 0 or not pair_psum), stop=False,
                    )
                for c in range(SP):
                    ch = blk * SP + c
                    # A carries the value: A[s,p] = (pi[s]==p) * val[s]
                    A = abpool.tile([128, 128], bf16, tag="A")
                    a_eng = nc.gpsimd if a_engine == "gpsimd" else nc.vector
                    a_eng.tensor_scalar(
                        A[:], iotaP[:], piT[:, ch, rl:rl + 1],
                        scaledT[:, ch, rl:rl + 1], Alu.is_equal, Alu.mult,
                    )
                    # B is the pure one-hot of fi (1-op, wide)
                    Bt = abpool.tile([128, FD], bf16, tag="B")
                    nc.vector.tensor_scalar(
                        Bt[:], iotaF[:], fiT[:, ch, rl:rl + 1], None,
                        Alu.is_equal,
                    )
                    last = (c == SP - 1) and (
                        not (pair_psum and mode == "diag") or j % 2 == 1
                    )
                    nc.tensor.matmul(
                        ps, lhsT=A[:], rhs=Bt[:],
                        start=(False if mode == "diag" else c == 0),
                        stop=last,
                    )
                if mode == "diag":
                    if pair_psum:
                        if j % 2 == 1:
                            nc.scalar.copy(
                                ot[:, j - 1:j + 1, :], psb[:, :, 0:FD]
                            )
                    else:
                        nc.scalar.copy(ot[:, j, :], ps)
                elif mode == "dma":
                    nc.vector.tensor_tensor(
                        ot[:, j, :], ot[:, j, :], ps[:], op=Alu.add
                    )
                else:
                    nc.vector.tensor_scalar(
                        ot[:, j, :], vt[:, j, :], pg_sc, None, Alu.mult
                    )
                    nc.vector.tensor_tensor(
                        ot[:, j, :], ot[:, j, :], ps[:], op=Alu.add
                    )
            nc.scalar.dma_start(out_v[:, gr, :], ot[:])

        for _ in range(rep):
            _phase1a()
            _phase1b()
            _phase2()

    nc.compile()
    _PROGRAM_CACHE[key] = nc
    return nc


def build_program_v4(R=R_FULL, FD=FD_FULL, SP=SP_FULL, G=8, a_pool=1,
                     d_eng="pool", out_dt="f32", rep=1):
    """v4: fp32r identity-diag base matmul (free dim padded to 256 so fp32r
    runs at 1 cycle/row), scatter one-hot builds split DVE/Pool, PSUM->SBUF
    copy on ACT, store from SBUF. out_dt="bf16" halves store traffic (host
    upcasts; scatter still accumulates in f32 PSUM).

    a_pool: how many of the 4 per-row A one-hot builds go to Pool (rest DVE).
    d_eng: engine for the per-row D = p_gen * I build ("pool" or "act").
    """
    key = ("v4", R, FD, SP, G, a_pool, d_eng, out_dt, rep)
    if key in _PROGRAM_CACHE:
        return _PROGRAM_CACHE[key]

    from contextlib import ExitStack

    import concourse.bass as bass
    import concourse.tile as tile
    from concourse import bacc, mybir

    f32 = mybir.dt.float32
    f32r = mybir.dt.float32r
    bf16 = mybir.dt.bfloat16
    Alu = mybir.AluOpType
    Act = mybir.ActivationFunctionType
    VL = 128 * FD
    RB = R // 128
    NG = R // G
    FP = 256  # padded free dim for the fp32r diag matmul
    assert R % 128 == 0 and R % G == 0 and G % 2 == 0

    nc = bacc.Bacc("TRN2", target_bir_lowering=False, debug=False)

    ctx_d = nc.dram_tensor("ctx", [R, H], f32, kind="ExternalInput")
    hid_d = nc.dram_tensor("hid", [R, H], f32, kind="ExternalInput")
    trg_d = nc.dram_tensor("trg", [R, H], f32, kind="ExternalInput")
    vocab_d = nc.dram_tensor("vocab", [R, VL], f32, kind="ExternalInput")
    attnT_d = nc.dram_tensor("attnT", [128, RB * SP, 128], f32, kind="ExternalInput")
    piT_d = nc.dram_tensor("piT", [128, RB * SP, 128], f32, kind="ExternalInput")
    fiT_d = nc.dram_tensor("fiT", [128, RB * SP, 128], f32, kind="ExternalInput")
    wh_d = nc.dram_tensor("wh", [128, H], f32, kind="ExternalInput")
    ws_d = nc.dram_tensor("ws", [128, H], f32, kind="ExternalInput")
    wx_d = nc.dram_tensor("wx", [128, H], f32, kind="ExternalInput")
    wxb_d = nc.dram_tensor("wxb", [128, 1], f32, kind="ExternalInput")
    iotaP_d = nc.dram_tensor("iotaP", [128, 128], bf16, kind="ExternalInput")
    iotaF_d = nc.dram_tensor("iotaF", [128, FD], bf16, kind="ExternalInput")
    ident_d = nc.dram_tensor("ident", [128, 128], f32, kind="ExternalInput")
    odt = f32 if out_dt == "f32" else bf16
    out_d = nc.dram_tensor("out", [R, VL], odt, kind="ExternalOutput")

    with tile.TileContext(nc) as tc, ExitStack() as es:
        singles = es.enter_context(tc.tile_pool(name="singles", bufs=1))
        ph1 = es.enter_context(tc.tile_pool(name="ph1", bufs=2))
        vpool = es.enter_context(tc.tile_pool(name="vpool", bufs=3))
        opool = es.enter_context(tc.tile_pool(name="opool", bufs=3))
        abpool = es.enter_context(tc.tile_pool(name="ab", bufs=6))
        dqpool = es.enter_context(tc.tile_pool(name="dq", bufs=4))
        ppool = es.enter_context(tc.tile_pool(name="psum", bufs=8, space="PSUM"))
        dpool = es.enter_context(tc.tile_pool(name="dram", bufs=1, space="DRAM"))

        attnT = singles.tile([128, RB * SP, 128], f32)
        nc.sync.dma_start(attnT[:], attnT_d[:])
        piT = singles.tile([128, RB * SP, 128], f32)
        nc.sync.dma_start(piT[:], piT_d[:])
        fiT = singles.tile([128, RB * SP, 128], f32)
        nc.sync.dma_start(fiT[:], fiT_d[:])
        iotaP = singles.tile([128, 128], bf16)
        nc.sync.dma_start(iotaP[:], iotaP_d[:])
        iotaF = singles.tile([128, FD], bf16)
        nc.sync.dma_start(iotaF[:], iotaF_d[:])
        ident = singles.tile([128, 128], f32)
        nc.sync.dma_start(ident[:], ident_d[:])
        wh = singles.tile([128, H], f32)
        nc.sync.dma_start(wh[:], wh_d[:])
        ws = singles.tile([128, H], f32)
        nc.sync.dma_start(ws[:], ws_d[:])
        wx = singles.tile([128, H], f32)
        nc.sync.dma_start(wx[:], wx_d[:])
        wxb = singles.tile([128, 1], f32)
        nc.sync.dma_start(wxb[:], wxb_d[:])
        scaledT = singles.tile([128, RB * SP, 128], f32)
        pgen_all = singles.tile([128, R], f32)
        om_all = singles.tile([128, R], f32)
        pgen_dram = dpool.tile([R, 1], f32)

        def _phase1a():
          for blk in range(RB):
            rows = slice(blk * 128, (blk + 1) * 128)
            gacc = ph1.tile([128, 1], f32, tag="gacc")
            gtmp = ph1.tile([128, 1], f32, tag="gtmp")
            g2 = ph1.tile([128, 1], f32, tag="g2")
            prod = ph1.tile([128, H], f32, tag="prod")
            for i, (src_d, w) in enumerate(
                ((ctx_d, wh), (hid_d, ws), (trg_d, wx))
            ):
                x = ph1.tile([128, H], f32, tag="x")
                nc.sync.dma_start(x[:], src_d[rows, :])
                nc.vector.tensor_tensor(prod[:], x[:], w[:], op=Alu.mult)
                dst = (gacc, gtmp, g2)[i]
                nc.vector.tensor_reduce(
                    dst[:], prod[:], axis=mybir.AxisListType.X, op=Alu.add
                )
            gsum = ph1.tile([128, 1], f32, tag="gsum")
            nc.vector.tensor_tensor(gsum[:], gacc[:], gtmp[:], op=Alu.add)
            gall = ph1.tile([128, 1], f32, tag="gall")
            nc.vector.tensor_tensor(gall[:], gsum[:], g2[:], op=Alu.add)
            pgen_col = ph1.tile([128, 1], f32, tag="pgen")
            nc.scalar.activation(
                pgen_col[:], gall[:], Act.Sigmoid, bias=wxb[:], scale=1.0
            )
            nc.sync.dma_start(pgen_dram[rows, :], pgen_col[:])

        def _phase1b():
            pg_flat = pgen_dram[:, 0]
            pg_bcast = bass.AP(
                tensor=pg_flat.tensor, offset=pg_flat.offset,
                ap=[[0, 128]] + list(pg_flat.ap),
            )
            nc.gpsimd.dma_start(pgen_all[:], pg_bcast)
            nc.vector.tensor_scalar(
                om_all[:], pgen_all[:], -1.0, 1.0, Alu.mult, Alu.add
            )
            for blk in range(RB):
                for c in range(SP):
                    nc.vector.tensor_tensor(
                        scaledT[:, blk * SP + c, :],
                        attnT[:, blk * SP + c, :],
                        om_all[:, blk * 128:(blk + 1) * 128],
                        op=Alu.mult,
                    )

        vocab_v = vocab_d[:].rearrange("r (p f) -> p r f", p=128)
        out_v = out_d[:].rearrange("r (p f) -> p r f", p=128)

        def _phase2():
          for grp in range(NG):
            gr = slice(grp * G, (grp + 1) * G)
            vt = vpool.tile([128, G, FP], f32)
            nc.gpsimd.memset(vt[:, :, FD:FP], 0.0)
            nc.sync.dma_start(vt[:, :, 0:FD], vocab_v[:, gr, :])
            ot = opool.tile([128, G, FD], odt)
            for j in range(G):
                r = grp * G + j
                blk = r // 128
                rl = r % 128
                pg_sc = pgen_all[:, r:r + 1]
                if j % 2 == 0:
                    psb = ppool.tile([128, 2, FP], f32)
                ps = psb[:, j % 2, :]
                D = dqpool.tile([128, 128], f32, tag="D")
                if d_eng == "pool":
                    nc.gpsimd.tensor_scalar(
                        D[:], ident[:], pg_sc, None, Alu.mult
                    )
                else:
                    nc.scalar.mul(D[:], ident[:], pg_sc)
                nc.tensor.matmul(
                    ps, lhsT=D[:].bitcast(f32r), rhs=vt[:, j, :].bitcast(f32r),
                    start=True, stop=False,
                )
                for c in range(SP):
                    ch = blk * SP + c
                    A = abpool.tile([128, 128], bf16, tag="A")
                    a_e = nc.gpsimd if c < a_pool else nc.vector
                    a_e.tensor_scalar(
                        A[:], iotaP[:], piT[:, ch, rl:rl + 1],
                        scaledT[:, ch, rl:rl + 1], Alu.is_equal, Alu.mult,
                    )
                    Bt = abpool.tile([128, FD], bf16, tag="B")
                    nc.vector.tensor_scalar(
                        Bt[:], iotaF[:], fiT[:, ch, rl:rl + 1], None,
                        Alu.is_equal,
                    )
                    nc.tensor.matmul(
                        ps[:, 0:FD], lhsT=A[:], rhs=Bt[:],
                        start=False, stop=(c == SP - 1),
                    )
                if j % 2 == 1:
                    nc.scalar.copy(ot[:, j - 1:j + 1, :], psb[:, :, 0:FD])
            nc.scalar.dma_start(out_v[:, gr, :], ot[:])

        for _ in range(rep):
            _phase1a()
            _phase1b()
            _phase2()

    nc.compile()
    _PROGRAM_CACHE[key] = nc
    return nc


K_LS = 16   # local_scatter bucket capacity per (row, partition)


def build_program_v5(R=R_FULL, FD=FD_FULL, G=8, K=K_LS, LC=1, out_dt="f32",
                     base="f32", rep=1):
    """v5: scatter via GPSIMD local_scatter.

    Host pre-buckets each row's 400 (p=v//250, f=v%250, val) triples by
    target partition p (pure integer/data-movement). Device per row:
      - DVE scales the bucketed raw attn values by (1-p_gen)  [128,K] bf16
      - Pool local_scatter places them: scat[p, f] = val       [128,250] bf16
      - PE: ps = (pgen*I) @ vocab_row  (fp32 diag, starts PSUM group)
            ps += I_bf16 @ scat        (identity merge)
            ps += A_l @ B_l            (one leftover one-hot chunk: intra-row
                                        duplicate (p,f) pairs + bucket
                                        overflow; usually ~2 entries)
      - ACT copies PSUM->SBUF (pairs), group store from SBUF.
    """
    key = ("v5", R, FD, G, K, LC, out_dt, base, rep)
    if key in _PROGRAM_CACHE:
        return _PROGRAM_CACHE[key]

    from contextlib import ExitStack

    import concourse.bass as bass
    import concourse.tile as tile
    from concourse import bacc, mybir

    f32 = mybir.dt.float32
    bf16 = mybir.dt.bfloat16
    i16 = mybir.dt.int16
    Alu = mybir.AluOpType
    Act = mybir.ActivationFunctionType
    VL = 128 * FD
    RB = R // 128
    NG = R // G
    assert R % 128 == 0 and R % G == 0 and G % 2 == 0

    nc = bacc.Bacc("TRN2", target_bir_lowering=False, debug=False)

    vdt = f32 if base == "f32" else bf16
    ctx_d = nc.dram_tensor("ctx", [R, H], f32, kind="ExternalInput")
    hid_d = nc.dram_tensor("hid", [R, H], f32, kind="ExternalInput")
    trg_d = nc.dram_tensor("trg", [R, H], f32, kind="ExternalInput")
    # vocab/out live transposed in DRAM ([p, r, f], host pre/post transpose)
    # so each group DMA is one large contiguous descriptor per partition.
    vocab_d = nc.dram_tensor("vocab", [128, R, FD], vdt, kind="ExternalInput")
    dls_d = nc.dram_tensor("dls", [128, R, K], bf16, kind="ExternalInput")
    ils_d = nc.dram_tensor("ils", [128, R, K], i16, kind="ExternalInput")
    attnL_d = nc.dram_tensor("attnL", [128, LC * RB, 128], f32, kind="ExternalInput")
    piL_d = nc.dram_tensor("piL", [128, LC * RB, 128], f32, kind="ExternalInput")
    fiL_d = nc.dram_tensor("fiL", [128, LC * RB, 128], f32, kind="ExternalInput")
    wh_d = nc.dram_tensor("wh", [128, H], f32, kind="ExternalInput")
    ws_d = nc.dram_tensor("ws", [128, H], f32, kind="ExternalInput")
    wx_d = nc.dram_tensor("wx", [128, H], f32, kind="ExternalInput")
    wxb_d = nc.dram_tensor("wxb", [128, 1], f32, kind="ExternalInput")
    iotaP_d = nc.dram_tensor("iotaP", [128, 128], bf16, kind="ExternalInput")
    iotaF_d = nc.dram_tensor("iotaF", [128, FD], bf16, kind="ExternalInput")
    ident_d = nc.dram_tensor("ident", [128, 128], f32, kind="ExternalInput")
    identb_d = nc.dram_tensor("identb", [128, 128], bf16, kind="ExternalInput")
    odt = f32 if out_dt == "f32" else bf16
    out_d = nc.dram_tensor("out", [128, R, FD], odt, kind="ExternalOutput")

    with tile.TileContext(nc) as tc, ExitStack() as es:
        singles = es.enter_context(tc.tile_pool(name="singles", bufs=1))
        ph1 = es.enter_context(tc.tile_pool(name="ph1", bufs=2))
        vpool = es.enter_context(tc.tile_pool(name="vpool", bufs=6))
        opool = es.enter_context(tc.tile_pool(name="opool", bufs=3))
        abpool = es.enter_context(tc.tile_pool(name="ab", bufs=6))
        scpool = es.enter_context(tc.tile_pool(name="sc", bufs=6))
        ppool = es.enter_context(tc.tile_pool(name="psum", bufs=8, space="PSUM"))
        dpool = es.enter_context(tc.tile_pool(name="dram", bufs=1, space="DRAM"))

        dls = singles.tile([128, R, K], bf16)
        nc.sync.dma_start(dls[:], dls_d[:])
        ils = singles.tile([128, R, K], i16)
        nc.sync.dma_start(ils[:], ils_d[:])
        attnL = singles.tile([128, LC * RB, 128], f32)
        nc.sync.dma_start(attnL[:], attnL_d[:])
        piL = singles.tile([128, LC * RB, 128], f32)
        nc.sync.dma_start(piL[:], piL_d[:])
        fiL = singles.tile([128, LC * RB, 128], f32)
        nc.sync.dma_start(fiL[:], fiL_d[:])
        iotaP = singles.tile([128, 128], bf16)
        nc.sync.dma_start(iotaP[:], iotaP_d[:])
        iotaF = singles.tile([128, FD], bf16)
        nc.sync.dma_start(iotaF[:], iotaF_d[:])
        ident = singles.tile([128, 128], f32)
        nc.sync.dma_start(ident[:], ident_d[:])
        identb = singles.tile([128, 128], bf16)
        nc.sync.dma_start(identb[:], identb_d[:])
        wh = singles.tile([128, H], f32)
        nc.sync.dma_start(wh[:], wh_d[:])
        ws = singles.tile([128, H], f32)
        nc.sync.dma_start(ws[:], ws_d[:])
        wx = singles.tile([128, H], f32)
        nc.sync.dma_start(wx[:], wx_d[:])
        wxb = singles.tile([128, 1], f32)
        nc.sync.dma_start(wxb[:], wxb_d[:])
        scaledL = singles.tile([128, LC * RB, 128], f32)
        pgen_all = singles.tile([128, R], f32)
        om_all = singles.tile([128, R], f32)
        pgen_dram = dpool.tile([R, 1], f32)

        def _phase1a():
          for blk in range(RB):
            rows = slice(blk * 128, (blk + 1) * 128)
            gacc = ph1.tile([128, 1], f32, tag="gacc")
            gtmp = ph1.tile([128, 1], f32, tag="gtmp")
            g2 = ph1.tile([128, 1], f32, tag="g2")
            prod = ph1.tile([128, H], f32, tag="prod")
            for i, (src_d, w) in enumerate(
                ((ctx_d, wh), (hid_d, ws), (trg_d, wx))
            ):
                x = ph1.tile([128, H], f32, tag="x")
                nc.sync.dma_start(x[:], src_d[rows, :])
                nc.vector.tensor_tensor(prod[:], x[:], w[:], op=Alu.mult)
                dst = (gacc, gtmp, g2)[i]
                nc.vector.tensor_reduce(
                    dst[:], prod[:], axis=mybir.AxisListType.X, op=Alu.add
                )
            gsum = ph1.tile([128, 1], f32, tag="gsum")
            nc.vector.tensor_tensor(gsum[:], gacc[:], gtmp[:], op=Alu.add)
            gall = ph1.tile([128, 1], f32, tag="gall")
            nc.vector.tensor_tensor(gall[:], gsum[:], g2[:], op=Alu.add)
            pgen_col = ph1.tile([128, 1], f32, tag="pgen")
            nc.scalar.activation(
                pgen_col[:], gall[:], Act.Sigmoid, bias=wxb[:], scale=1.0
            )
            nc.sync.dma_start(pgen_dram[rows, :], pgen_col[:])

        def _phase1b():
            pg_flat = pgen_dram[:, 0]
            pg_bcast = bass.AP(
                tensor=pg_flat.tensor, offset=pg_flat.offset,
                ap=[[0, 128]] + list(pg_flat.ap),
            )
            nc.gpsimd.dma_start(pgen_all[:], pg_bcast)
            nc.vector.tensor_scalar(
                om_all[:], pgen_all[:], -1.0, 1.0, Alu.mult, Alu.add
            )
            for lb in range(LC * RB):
                blk = lb % RB
                nc.vector.tensor_tensor(
                    scaledL[:, lb, :],
                    attnL[:, lb, :],
                    om_all[:, blk * 128:(blk + 1) * 128],
                    op=Alu.mult,
                )

        vocab_v = vocab_d[:]
        out_v = out_d[:]

        vt_pre = {}

        def _preload(grp):
            gr = slice(grp * G, (grp + 1) * G)
            vt = vpool.tile([128, G, FD], vdt)
            nc.sync.dma_start(vt[:], vocab_v[:, gr, :])
            vt_pre[grp] = vt

        def _phase2():
          for grp in range(NG):
            gr = slice(grp * G, (grp + 1) * G)
            if grp in vt_pre:
                vt = vt_pre.pop(grp)
            else:
                vt = vpool.tile([128, G, FD], vdt)
                nc.sync.dma_start(vt[:], vocab_v[:, gr, :])
            ot = opool.tile([128, G, FD], odt)
            for j in range(G):
                r = grp * G + j
                blk = r // 128
                rl = r % 128
                pg_sc = pgen_all[:, r:r + 1]
                om_sc = om_all[:, r:r + 1]
                if j % 2 == 0:
                    psb = ppool.tile([128, 2, FD], f32)
                ps = psb[:, j % 2, :]
                # base: (pgen * I) @ vocab_row, starts the PSUM accum group
                if base == "f32":
                    D = abpool.tile([128, 128], f32, tag="D")
                    nc.vector.tensor_scalar(D[:], ident[:], pg_sc, None,
                                            Alu.mult)
                else:
                    D = abpool.tile([128, 128], bf16, tag="D")
                    nc.vector.tensor_scalar(D[:], identb[:], pg_sc, None,
                                            Alu.mult)
                nc.tensor.matmul(ps, lhsT=D[:], rhs=vt[:, j, :],
                                 start=True, stop=False)
                # scatter: scale bucketed values, local_scatter, identity-merge
                sval = scpool.tile([128, K], bf16, tag="sval")
                nc.vector.tensor_scalar(sval[:], dls[:, r, :], om_sc, None,
                                        Alu.mult)
                scat = scpool.tile([128, FD], bf16, tag="scat")
                nc.gpsimd.local_scatter(
                    scat[:], sval[:], ils[:, r, :],
                    channels=128, num_elems=FD, num_idxs=K,
                )
                nc.tensor.matmul(ps, lhsT=identb[:], rhs=scat[:],
                                 start=False, stop=False)
                # leftover chunks (duplicates/overflow): one-hot matmuls
                for l in range(LC):
                    lb = l * RB + blk
                    A = abpool.tile([128, 128], bf16, tag="A")
                    nc.vector.tensor_scalar(
                        A[:], iotaP[:], piL[:, lb, rl:rl + 1],
                        scaledL[:, lb, rl:rl + 1], Alu.is_equal, Alu.mult,
                    )
                    Bt = abpool.tile([128, FD], bf16, tag="B")
                    nc.vector.tensor_scalar(
                        Bt[:], iotaF[:], fiL[:, lb, rl:rl + 1], None,
                        Alu.is_equal,
                    )
                    nc.tensor.matmul(ps, lhsT=A[:], rhs=Bt[:],
                                     start=False, stop=(l == LC - 1))
                if j % 2 == 1:
                    nc.scalar.copy(ot[:, j - 1:j + 1, :], psb[:, :, :])
            nc.scalar.dma_start(out_v[:, gr, :], ot[:])

        for _ in range(rep):
            for g in range(4):
                _preload(g)
            _phase1a()
            _phase1b()
            _phase2()

    nc.compile()
    _PROGRAM_CACHE[key] = nc
    return nc


def make_core_inputs(ctx, hid, trg, vocab, attn, ids, w_h, w_s, w_x_w, w_x_b,
                     R=R_FULL, FD=FD_FULL, SP=SP_FULL):
    """Host-side prep for one core: flatten rows, decompose + transpose indices.

    ctx/hid/trg: [R, H] f32; vocab: [R, 128*FD] f32; attn: [R, S'] f32;
    ids: [R, S'] int. Returns the in_map dict for this core.
    """
    RB = R // 128
    Sp = SP * 128
    Sl = attn.shape[1]
    f32 = np.float32

    ids = np.asarray(ids).astype(np.int64)
    pi = (ids // FD).astype(f32)
    fi = (ids % FD).astype(f32)

    def tr(x, pad):
        full = np.full((R, Sp), pad, dtype=f32)
        full[:, :Sl] = x
        # [R, Sp] -> [RB, 128(r), SP, 128(s)] -> [s, RB, SP, r]
        t = full.reshape(RB, 128, SP, 128).transpose(3, 0, 2, 1)
        return np.ascontiguousarray(t.reshape(128, RB * SP, 128))

    def rep(w, n):
        return np.ascontiguousarray(
            np.broadcast_to(np.asarray(w, dtype=f32).reshape(1, n), (128, n))
        )

    return {
        "ctx": np.ascontiguousarray(ctx, dtype=f32),
        "hid": np.ascontiguousarray(hid, dtype=f32),
        "trg": np.ascontiguousarray(trg, dtype=f32),
        "vocab": np.ascontiguousarray(vocab, dtype=f32),
        "attnT": tr(np.asarray(attn, dtype=f32), 0.0),
        "piT": tr(pi, 1.0e4),
        "fiT": tr(fi, -1.0),
        "wh": rep(w_h, H),
        "ws": rep(w_s, H),
        "wx": rep(w_x_w, H),
        "wxb": rep(w_x_b, 1),
        "iotaP": rep(np.arange(128, dtype=f32), 128).astype(bfloat16),
        "iotaF": rep(np.arange(FD, dtype=f32), FD).astype(bfloat16),
        "ident": np.eye(128, dtype=f32),
    }


def make_core_inputs_v5(ctx, hid, trg, vocab, attn, ids, w_h, w_s, w_x_w,
                        w_x_b, R=R_FULL, FD=FD_FULL, K=K_LS):
    """Host prep for one core, v5 layout: bucket each row's (p=v//FD,
    f=v%FD, val) triples by target partition p. Integer index work plus
    value placement only — all arithmetic on the values happens on device.

    Layer 1 (local_scatter): first occurrence of each (row, p, f), up to K
    per (row, p). Everything else (duplicate (p,f) pairs, bucket overflow)
    goes to one leftover one-hot chunk per row (capacity 128).
    """
    RB = R // 128
    Sl = ids.shape[1]
    f32 = np.float32

    ids = np.asarray(ids).astype(np.int64)
    attn = np.asarray(attn, dtype=f32)
    NS = R * Sl
    rr = np.repeat(np.arange(R), Sl)
    pp = (ids // FD).ravel()
    ff = (ids % FD).ravel()
    vv = attn.ravel()

    order = np.lexsort((ff, pp, rr))
    rs, ps, fs, vs = rr[order], pp[order], ff[order], vv[order]
    idx = np.arange(NS)
    new_rpf = np.r_[True, (rs[1:] != rs[:-1]) | (ps[1:] != ps[:-1])
                    | (fs[1:] != fs[:-1])]
    new_rp = np.r_[True, (rs[1:] != rs[:-1]) | (ps[1:] != ps[:-1])]
    new_r = np.r_[True, rs[1:] != rs[:-1]]
    keep = new_rpf
    kc0 = np.cumsum(keep) - keep          # kept strictly before element
    rp_start = np.maximum.accumulate(np.where(new_rp, idx, -1))
    rank = kc0 - kc0[rp_start]            # rank among kept within (r, p)
    layer1 = keep & (rank < K)

    data_ls = np.zeros((128, R, K), dtype=bfloat16)
    idx_ls = np.full((128, R, K), -1, dtype=np.int16)
    m = layer1
    data_ls[ps[m], rs[m], rank[m]] = vs[m].astype(bfloat16)
    idx_ls[ps[m], rs[m], rank[m]] = fs[m].astype(np.int16)

    lm = ~layer1
    lc0 = np.cumsum(lm) - lm
    r_start = np.maximum.accumulate(np.where(new_r, idx, -1))
    lslot = (lc0 - lc0[r_start])[lm]
    LC = 1 if lslot.size == 0 else int(lslot.max()) // 128 + 1
    lr = rs[lm]
    attnL = np.zeros((128, LC, RB, 128), dtype=f32)
    piL = np.full((128, LC, RB, 128), 1.0e4, dtype=f32)
    fiL = np.full((128, LC, RB, 128), -1.0, dtype=f32)
    attnL[lslot % 128, lslot // 128, lr // 128, lr % 128] = vs[lm]
    piL[lslot % 128, lslot // 128, lr // 128, lr % 128] = ps[lm].astype(f32)
    fiL[lslot % 128, lslot // 128, lr // 128, lr % 128] = fs[lm].astype(f32)
    attnL = attnL.reshape(128, LC * RB, 128)
    piL = piL.reshape(128, LC * RB, 128)
    fiL = fiL.reshape(128, LC * RB, 128)

    def rep(w, n):
        return np.ascontiguousarray(
            np.broadcast_to(np.asarray(w, dtype=f32).reshape(1, n), (128, n))
        )

    return {
        "ctx": np.ascontiguousarray(ctx, dtype=f32),
        "hid": np.ascontiguousarray(hid, dtype=f32),
        "trg": np.ascontiguousarray(trg, dtype=f32),
        "vocab": np.ascontiguousarray(
            np.asarray(vocab).reshape(R, 128, FD).transpose(1, 0, 2).astype(
                f32 if V5_KW.get("base", "f32") == "f32" else bfloat16)),
        "dls": data_ls,
        "ils": idx_ls,
        "attnL": attnL,
        "piL": piL,
        "fiL": fiL,
        "wh": rep(w_h, H),
        "ws": rep(w_s, H),
        "wx": rep(w_x_w, H),
        "wxb": rep(w_x_b, 1),
        "iotaP": rep(np.arange(128, dtype=f32), 128).astype(bfloat16),
        "iotaF": rep(np.arange(FD, dtype=f32), FD).astype(bfloat16),
        "ident": np.eye(128, dtype=f32),
        "identb": np.eye(128, dtype=f32).astype(bfloat16),
    }


def make_in_maps(context_vecs, hidden, trg_embs, vocab_dists, attn_dists,
                 src_ids, w_h, w_s, w_x_w, w_x_b):
    """Build the 8 per-core input dicts from full inputs."""
    context_vecs = np.asarray(context_vecs)
    hidden = np.asarray(hidden)
    trg_embs = np.asarray(trg_embs)
    vocab_dists = np.asarray(vocab_dists)
    attn_dists = np.asarray(attn_dists)
    src_ids = np.asarray(src_ids)

    mk = make_core_inputs_v5 if VARIANT == "v5" else make_core_inputs
    in_maps = []
    for i in range(N_CORES):
        bs = slice(i * BPC, (i + 1) * BPC)
        in_maps.append(mk(
            context_vecs[bs].reshape(R_FULL, H),
            hidden[bs].reshape(R_FULL, H),
            trg_embs[bs].reshape(R_FULL, H),
            vocab_dists[bs].reshape(R_FULL, V),
            attn_dists[bs].reshape(R_FULL, S),
            src_ids[bs].reshape(R_FULL, S),
            w_h, w_s, w_x_w, w_x_b,
        ))
    if VARIANT == "v5":
        # all cores must share one program: pad leftover chunks to max LC
        RB = R_FULL // 128
        lc_max = max(m["piL"].shape[1] // RB for m in in_maps)
        for m in in_maps:
            lc = m["piL"].shape[1] // RB
            if lc < lc_max:
                pad = ((0, 0), (0, (lc_max - lc) * RB), (0, 0))
                m["attnL"] = np.pad(m["attnL"], pad)
                m["piL"] = np.pad(m["piL"], pad, constant_values=1.0e4)
                m["fiL"] = np.pad(m["fiL"], pad, constant_values=-1.0)
        global _LAST_LC
        _LAST_LC = lc_max
    return in_maps


VARIANT = "v5"          # "v5", "v4", or "diag" (previous baseline)
V4_KW = dict(G=8, a_pool=1, d_eng="pool", out_dt="f32")
V5_KW = dict(G=8, out_dt="f32", base="bf16")
_LAST_LC = 1            # leftover-chunk count of the last make_in_maps


def build_current(rep=1):
    if VARIANT == "v5":
        return build_program_v5(rep=rep, LC=_LAST_LC, **V5_KW)
    if VARIANT == "v4":
        return build_program_v4(rep=rep, **V4_KW)
    return build_program(rep=rep)


def kernel(context_vecs, hidden, trg_embs, vocab_dists, attn_dists,
           src_ids, pad_id, w_h, w_s, w_x_w, w_x_b):
    """Full-input entry point. Shards over 8 NeuronCores, returns [B,T,V] f32."""
    from concourse.bass_utils import run_bass_kernel_spmd

    in_maps = make_in_maps(context_vecs, hidden, trg_embs, vocab_dists,
                           attn_dists, src_ids, w_h, w_s, w_x_w, w_x_b)
    nc = build_current()
    res = run_bass_kernel_spmd(nc, in_maps, list(range(N_CORES)))
    outs = []
    for i in range(N_CORES):
        o = np.asarray(res.results[i]["out"]).astype(np.float32)
        if VARIANT == "v5":
            # device layout [128, R, FD] -> [R, 128*FD]
            o = o.transpose(1, 0, 2).reshape(R_FULL, V)
        outs.append(o.reshape(BPC, T, V))
    return np.concatenate(outs, axis=0)



# revision 38
# speedup vs baseline: 1.8947x; 1.8947x over previous
"""CopyMechanism (pointer-generator) kernel for 8 Trainium2 NeuronCores.

Full problem: B=16, T=128, H=512, V=32000, S=400.
  gen = sigmoid(ctx@wh + hid@ws + trg@wx + b)          [B,T,1]
  out = gen * vocab_dists; out[b,t,ids[b,t,s]] += (1-gen)*attn[b,t,s]

Sharding: data-parallel over batch. Core i handles batches [2i, 2i+1]
(256 rows of T-steps). Weights replicated. No cross-core communication.

Device algorithm (per core, per row r):
  Decompose vocab index v = p*250 + f  (V = 128*250), so a row's 32000-wide
  output is an SBUF tile [128 partitions, 250 free].  The scatter-add of the
  S=400 attn values becomes a sum of outer products:
     M[p,f] = sum_s onehot(pi[s])[p] * (onehot(fi[s])[f] * val[s])
  computed by TensorE matmuls contracting s (4 chunks of <=128 on the
  partition axis).  One-hots are built on VectorE with iota/is_equal
  tensor_scalar ops in bf16 (indices pre-decomposed and pre-transposed on
  host -- integer-only preprocessing); A carries the scatter value.  The
  base p_gen*vocab is a 5th fp32 matmul with lhsT = p_gen*I (diagonal,
  built per row on ScalarE) that starts the PSUM accumulation group;
  ScalarE copies PSUM->SBUF and the store goes out on the ACT HWDGE ring
  (loads on the SP ring) so loads and stores don't serialize on one FIFO.

  p_gen is computed on-device (dot products + sigmoid), bounced through a
  DRAM scratch and re-loaded with a partition-broadcast AP so each row's
  scalar is available on all 128 partitions.
"""

import numpy as np
from ml_dtypes import bfloat16

# ---------------------------------------------------------------------------
# problem constants (hardcoded per contract)
B, T, H, V, S = 16, 128, 512, 32000, 400
N_CORES = 8
BPC = B // N_CORES          # batches per core
R_FULL = BPC * T            # rows per core = 256
FD_FULL = V // 128          # 250
SP_FULL = (S + 127) // 128  # 4 s-chunks
G_FULL = 16                 # rows per vocab DMA group

_PROGRAM_CACHE = {}


def build_program(R=R_FULL, FD=FD_FULL, SP=SP_FULL, G=G_FULL, mode="diag",
                  rep=1, a_engine="dve", ablate="full", pair_psum=True):
    """Build + compile the per-core Bass program. Same program for all cores.

    R : rows per core (multiple of 128)
    FD: free-dim width of the vocab decomposition (V_local = 128*FD)
    SP: number of 128-wide s-chunks (S padded to SP*128)
    G : rows per vocab/out DMA group
    mode: "diag" -> base p_gen*vocab via a diagonal matmul starting the PSUM
          group; "dve" -> base+merge on VectorE after the scatter matmuls.
    rep : repeat the whole body rep times (identical output; used for
          differential device-time measurement).
    """
    key = (R, FD, SP, G, mode, rep, a_engine, ablate, pair_psum)
    if key in _PROGRAM_CACHE:
        return _PROGRAM_CACHE[key]

    from contextlib import ExitStack

    import concourse.bass as bass
    import concourse.tile as tile
    from concourse import bacc, mybir

    f32 = mybir.dt.float32
    bf16 = mybir.dt.bfloat16
    Alu = mybir.AluOpType
    Act = mybir.ActivationFunctionType
    VL = 128 * FD
    RB = R // 128
    NG = R // G
    assert R % 128 == 0 and R % G == 0

    nc = bacc.Bacc("TRN2", target_bir_lowering=False, debug=False)

    ctx_d = nc.dram_tensor("ctx", [R, H], f32, kind="ExternalInput")
    hid_d = nc.dram_tensor("hid", [R, H], f32, kind="ExternalInput")
    trg_d = nc.dram_tensor("trg", [R, H], f32, kind="ExternalInput")
    vocab_d = nc.dram_tensor("vocab", [R, VL], f32, kind="ExternalInput")
    attnT_d = nc.dram_tensor("attnT", [128, RB * SP, 128], f32, kind="ExternalInput")
    piT_d = nc.dram_tensor("piT", [128, RB * SP, 128], f32, kind="ExternalInput")
    fiT_d = nc.dram_tensor("fiT", [128, RB * SP, 128], f32, kind="ExternalInput")
    # weights replicated across partitions on host (pure data movement)
    wh_d = nc.dram_tensor("wh", [128, H], f32, kind="ExternalInput")
    ws_d = nc.dram_tensor("ws", [128, H], f32, kind="ExternalInput")
    wx_d = nc.dram_tensor("wx", [128, H], f32, kind="ExternalInput")
    wxb_d = nc.dram_tensor("wxb", [128, 1], f32, kind="ExternalInput")
    iotaP_d = nc.dram_tensor("iotaP", [128, 128], bf16, kind="ExternalInput")
    iotaF_d = nc.dram_tensor("iotaF", [128, FD], bf16, kind="ExternalInput")
    ident_d = nc.dram_tensor("ident", [128, 128], f32, kind="ExternalInput")
    out_d = nc.dram_tensor("out", [R, VL], f32, kind="ExternalOutput")

    with tile.TileContext(nc) as tc, ExitStack() as es:
        singles = es.enter_context(tc.tile_pool(name="singles", bufs=1))
        ph1 = es.enter_context(tc.tile_pool(name="ph1", bufs=2))
        gbufs = 3 if G <= 16 else 2
        vpool = es.enter_context(tc.tile_pool(name="vpool", bufs=gbufs))
        opool = es.enter_context(tc.tile_pool(name="opool", bufs=gbufs))
        abpool = es.enter_context(tc.tile_pool(name="ab", bufs=6))
        ppool = es.enter_context(tc.tile_pool(name="psum", bufs=8, space="PSUM"))
        dpool = es.enter_context(tc.tile_pool(name="dram", bufs=1, space="DRAM"))

        # --- constants / small inputs ---
        attnT = singles.tile([128, RB * SP, 128], f32)
        nc.sync.dma_start(attnT[:], attnT_d[:])
        piT = singles.tile([128, RB * SP, 128], f32)
        nc.sync.dma_start(piT[:], piT_d[:])
        fiT = singles.tile([128, RB * SP, 128], f32)
        nc.sync.dma_start(fiT[:], fiT_d[:])
        iotaP = singles.tile([128, 128], bf16)
        nc.sync.dma_start(iotaP[:], iotaP_d[:])
        iotaF = singles.tile([128, FD], bf16)
        nc.sync.dma_start(iotaF[:], iotaF_d[:])
        ident = singles.tile([128, 128], f32)
        nc.sync.dma_start(ident[:], ident_d[:])
        wh = singles.tile([128, H], f32)
        nc.sync.dma_start(wh[:], wh_d[:])
        ws = singles.tile([128, H], f32)
        nc.sync.dma_start(ws[:], ws_d[:])
        wx = singles.tile([128, H], f32)
        nc.sync.dma_start(wx[:], wx_d[:])
        wxb = singles.tile([128, 1], f32)
        nc.sync.dma_start(wxb[:], wxb_d[:])
        scaledT = singles.tile([128, RB * SP, 128], f32)
        pgen_all = singles.tile([128, R], f32)
        om_all = singles.tile([128, R], f32)
        pgen_dram = dpool.tile([R, 1], f32)

        # --- phase 1a: p_gen per row (rows on partitions), bounce to DRAM ---
        def _phase1a():
          for blk in range(RB):
            rows = slice(blk * 128, (blk + 1) * 128)
            gacc = ph1.tile([128, 1], f32, tag="gacc")
            gtmp = ph1.tile([128, 1], f32, tag="gtmp")
            g2 = ph1.tile([128, 1], f32, tag="g2")
            prod = ph1.tile([128, H], f32, tag="prod")
            for i, (src_d, w) in enumerate(
                ((ctx_d, wh), (hid_d, ws), (trg_d, wx))
            ):
                x = ph1.tile([128, H], f32, tag="x")
                nc.sync.dma_start(x[:], src_d[rows, :])
                nc.vector.tensor_tensor(prod[:], x[:], w[:], op=Alu.mult)
                dst = (gacc, gtmp, g2)[i]
                nc.vector.tensor_reduce(
                    dst[:], prod[:], axis=mybir.AxisListType.X, op=Alu.add
                )
            gsum = ph1.tile([128, 1], f32, tag="gsum")
            nc.vector.tensor_tensor(gsum[:], gacc[:], gtmp[:], op=Alu.add)
            gall = ph1.tile([128, 1], f32, tag="gall")
            nc.vector.tensor_tensor(gall[:], gsum[:], g2[:], op=Alu.add)
            pgen_col = ph1.tile([128, 1], f32, tag="pgen")
            nc.scalar.activation(
                pgen_col[:], gall[:], Act.Sigmoid, bias=wxb[:], scale=1.0
            )
            nc.sync.dma_start(pgen_dram[rows, :], pgen_col[:])

        # --- phase 1b: broadcast p_gen to all partitions; scaled attnT ---
        def _phase1b():
            pg_flat = pgen_dram[:, 0]
            pg_bcast = bass.AP(
                tensor=pg_flat.tensor, offset=pg_flat.offset,
                ap=[[0, 128]] + list(pg_flat.ap),
            )
            nc.gpsimd.dma_start(pgen_all[:], pg_bcast)
            nc.vector.tensor_scalar(
                om_all[:], pgen_all[:], -1.0, 1.0, Alu.mult, Alu.add
            )
            for blk in range(RB):
                for c in range(SP):
                    nc.vector.tensor_tensor(
                        scaledT[:, blk * SP + c, :],
                        attnT[:, blk * SP + c, :],
                        om_all[:, blk * 128:(blk + 1) * 128],
                        op=Alu.mult,
                    )

        # --- phase 2: per-row scatter-add via one-hot matmuls ---
        vocab_v = vocab_d[:].rearrange("r (p f) -> p r f", p=128)
        out_v = out_d[:].rearrange("r (p f) -> p r f", p=128)

        def _phase2():
          for grp in range(NG):
            gr = slice(grp * G, (grp + 1) * G)
            ot = opool.tile([128, G, FD], f32)
            if mode == "dma":
                # Pre-fill ot with per-row p_gen, then the vocab load DMA
                # multiplies in transit: ot = p_gen * vocab (no PE/DVE time).
                for j in range(G):
                    r = grp * G + j
                    # ot[:, j, :] = 0*iotaF + p_gen[r]  (no broadcast APs)
                    nc.scalar.activation(
                        ot[:, j, :], iotaF[:], Act.Identity,
                        bias=pgen_all[:, r:r + 1], scale=0.0,
                    )
                nc.gpsimd.dma_start(
                    ot[:], vocab_v[:, gr, :], accum_op=Alu.mult
                )
            else:
                vt = vpool.tile([128, G, FD], f32)
                nc.sync.dma_start(vt[:], vocab_v[:, gr, :])
            if ablate == "dmaonly":
                if mode != "dma":
                    nc.scalar.copy(ot[:, :, :], vt[:, :, :])
                nc.scalar.dma_start(out_v[:, gr, :], ot[:])
                continue
            psb = None
            for j in range(G):
                r = grp * G + j
                blk = r // 128
                rl = r % 128
                if ablate == "nomm":
                    nc.scalar.copy(ot[:, j, :], vt[:, j, :])
                if pair_psum and mode == "diag":
                    if j % 2 == 0:
                        psb = ppool.tile([128, 2, 256], f32)
                    ps = psb[:, j % 2, 0:FD]
                else:
                    ps = ppool.tile([128, FD], f32)[:]
                pg_sc = pgen_all[:, r:r + 1]
                if ablate == "nomm":
                    for c in range(SP):
                        ch = blk * SP + c
                        A = abpool.tile([128, 128], bf16, tag="A")
                        eng = nc.gpsimd if a_engine == "gpsimd" else nc.vector
                        eng.tensor_scalar(
                            A[:], iotaP[:], piT[:, ch, rl:rl + 1],
                            scaledT[:, ch, rl:rl + 1], Alu.is_equal, Alu.mult,
                        )
                        Bt = abpool.tile([128, FD], bf16, tag="B")
                        nc.vector.tensor_scalar(
                            Bt[:], iotaF[:], fiT[:, ch, rl:rl + 1], None,
                            Alu.is_equal,
                        )
                    continue
                if mode == "diag":
                    D = abpool.tile([128, 128], f32, tag="D")
                    nc.scalar.mul(D[:], ident[:], pg_sc)
                    nc.tensor.matmul(
                        ps, lhsT=D[:], rhs=vt[:, j, :],
                        start=(j % 2 == 0 or not pair_psum), stop=False,
                    )
                for c in range(SP):
                    ch = blk * SP + c
                    # A carries the value: A[s,p] = (pi[s]==p) * val[s]
                    A = abpool.tile([128, 128], bf16, tag="A")
                    a_eng = nc.gpsimd if a_engine == "gpsimd" else nc.vector
                    a_eng.tensor_scalar(
                        A[:], iotaP[:], piT[:, ch, rl:rl + 1],
                        scaledT[:, ch, rl:rl + 1], Alu.is_equal, Alu.mult,
                    )
                    # B is the pure one-hot of fi (1-op, wide)
                    Bt = abpool.tile([128, FD], bf16, tag="B")
                    nc.vector.tensor_scalar(
                        Bt[:], iotaF[:], fiT[:, ch, rl:rl + 1], None,
                        Alu.is_equal,
                    )
                    last = (c == SP - 1) and (
                        not (pair_psum and mode == "diag") or j % 2 == 1
                    )
                    nc.tensor.matmul(
                        ps, lhsT=A[:], rhs=Bt[:],
                        start=(False if mode == "diag" else c == 0),
                        stop=last,
                    )
                if mode == "diag":
                    if pair_psum:
                        if j % 2 == 1:
                            nc.scalar.copy(
                                ot[:, j - 1:j + 1, :], psb[:, :, 0:FD]
                            )
                    else:
                        nc.scalar.copy(ot[:, j, :], ps)
                elif mode == "dma":
                    nc.vector.tensor_tensor(
                        ot[:, j, :], ot[:, j, :], ps[:], op=Alu.add
                    )
                else:
                    nc.vector.tensor_scalar(
                        ot[:, j, :], vt[:, j, :], pg_sc, None, Alu.mult
                    )
                    nc.vector.tensor_tensor(
                        ot[:, j, :], ot[:, j, :], ps[:], op=Alu.add
                    )
            nc.scalar.dma_start(out_v[:, gr, :], ot[:])

        for _ in range(rep):
            _phase1a()
            _phase1b()
            _phase2()

    nc.compile()
    _PROGRAM_CACHE[key] = nc
    return nc


def build_program_v4(R=R_FULL, FD=FD_FULL, SP=SP_FULL, G=8, a_pool=1,
                     d_eng="pool", out_dt="f32", rep=1):
    """v4: fp32r identity-diag base matmul (free dim padded to 256 so fp32r
    runs at 1 cycle/row), scatter one-hot builds split DVE/Pool, PSUM->SBUF
    copy on ACT, store from SBUF. out_dt="bf16" halves store traffic (host
    upcasts; scatter still accumulates in f32 PSUM).

    a_pool: how many of the 4 per-row A one-hot builds go to Pool (rest DVE).
    d_eng: engine for the per-row D = p_gen * I build ("pool" or "act").
    """
    key = ("v4", R, FD, SP, G, a_pool, d_eng, out_dt, rep)
    if key in _PROGRAM_CACHE:
        return _PROGRAM_CACHE[key]

    from contextlib import ExitStack

    import concourse.bass as bass
    import concourse.tile as tile
    from concourse import bacc, mybir

    f32 = mybir.dt.float32
    f32r = mybir.dt.float32r
    bf16 = mybir.dt.bfloat16
    Alu = mybir.AluOpType
    Act = mybir.ActivationFunctionType
    VL = 128 * FD
    RB = R // 128
    NG = R // G
    FP = 256  # padded free dim for the fp32r diag matmul
    assert R % 128 == 0 and R % G == 0 and G % 2 == 0

    nc = bacc.Bacc("TRN2", target_bir_lowering=False, debug=False)

    ctx_d = nc.dram_tensor("ctx", [R, H], f32, kind="ExternalInput")
    hid_d = nc.dram_tensor("hid", [R, H], f32, kind="ExternalInput")
    trg_d = nc.dram_tensor("trg", [R, H], f32, kind="ExternalInput")
    vocab_d = nc.dram_tensor("vocab", [R, VL], f32, kind="ExternalInput")
    attnT_d = nc.dram_tensor("attnT", [128, RB * SP, 128], f32, kind="ExternalInput")
    piT_d = nc.dram_tensor("piT", [128, RB * SP, 128], f32, kind="ExternalInput")
    fiT_d = nc.dram_tensor("fiT", [128, RB * SP, 128], f32, kind="ExternalInput")
    wh_d = nc.dram_tensor("wh", [128, H], f32, kind="ExternalInput")
    ws_d = nc.dram_tensor("ws", [128, H], f32, kind="ExternalInput")
    wx_d = nc.dram_tensor("wx", [128, H], f32, kind="ExternalInput")
    wxb_d = nc.dram_tensor("wxb", [128, 1], f32, kind="ExternalInput")
    iotaP_d = nc.dram_tensor("iotaP", [128, 128], bf16, kind="ExternalInput")
    iotaF_d = nc.dram_tensor("iotaF", [128, FD], bf16, kind="ExternalInput")
    ident_d = nc.dram_tensor("ident", [128, 128], f32, kind="ExternalInput")
    odt = f32 if out_dt == "f32" else bf16
    out_d = nc.dram_tensor("out", [R, VL], odt, kind="ExternalOutput")

    with tile.TileContext(nc) as tc, ExitStack() as es:
        singles = es.enter_context(tc.tile_pool(name="singles", bufs=1))
        ph1 = es.enter_context(tc.tile_pool(name="ph1", bufs=2))
        vpool = es.enter_context(tc.tile_pool(name="vpool", bufs=3))
        opool = es.enter_context(tc.tile_pool(name="opool", bufs=3))
        abpool = es.enter_context(tc.tile_pool(name="ab", bufs=6))
        dqpool = es.enter_context(tc.tile_pool(name="dq", bufs=4))
        ppool = es.enter_context(tc.tile_pool(name="psum", bufs=8, space="PSUM"))
        dpool = es.enter_context(tc.tile_pool(name="dram", bufs=1, space="DRAM"))

        attnT = singles.tile([128, RB * SP, 128], f32)
        nc.sync.dma_start(attnT[:], attnT_d[:])
        piT = singles.tile([128, RB * SP, 128], f32)
        nc.sync.dma_start(piT[:], piT_d[:])
        fiT = singles.tile([128, RB * SP, 128], f32)
        nc.sync.dma_start(fiT[:], fiT_d[:])
        iotaP = singles.tile([128, 128], bf16)
        nc.sync.dma_start(iotaP[:], iotaP_d[:])
        iotaF = singles.tile([128, FD], bf16)
        nc.sync.dma_start(iotaF[:], iotaF_d[:])
        ident = singles.tile([128, 128], f32)
        nc.sync.dma_start(ident[:], ident_d[:])
        wh = singles.tile([128, H], f32)
        nc.sync.dma_start(wh[:], wh_d[:])
        ws = singles.tile([128, H], f32)
        nc.sync.dma_start(ws[:], ws_d[:])
        wx = singles.tile([128, H], f32)
        nc.sync.dma_start(wx[:], wx_d[:])
        wxb = singles.tile([128, 1], f32)
        nc.sync.dma_start(wxb[:], wxb_d[:])
        scaledT = singles.tile([128, RB * SP, 128], f32)
        pgen_all = singles.tile([128, R], f32)
        om_all = singles.tile([128, R], f32)
        pgen_dram = dpool.tile([R, 1], f32)

        def _phase1a():
          for blk in range(RB):
            rows = slice(blk * 128, (blk + 1) * 128)
            gacc = ph1.tile([128, 1], f32, tag="gacc")
            gtmp = ph1.tile([128, 1], f32, tag="gtmp")
            g2 = ph1.tile([128, 1], f32, tag="g2")
            prod = ph1.tile([128, H], f32, tag="prod")
            for i, (src_d, w) in enumerate(
                ((ctx_d, wh), (hid_d, ws), (trg_d, wx))
            ):
                x = ph1.tile([128, H], f32, tag="x")
                nc.sync.dma_start(x[:], src_d[rows, :])
                nc.vector.tensor_tensor(prod[:], x[:], w[:], op=Alu.mult)
                dst = (gacc, gtmp, g2)[i]
                nc.vector.tensor_reduce(
                    dst[:], prod[:], axis=mybir.AxisListType.X, op=Alu.add
                )
            gsum = ph1.tile([128, 1], f32, tag="gsum")
            nc.vector.tensor_tensor(gsum[:], gacc[:], gtmp[:], op=Alu.add)
            gall = ph1.tile([128, 1], f32, tag="gall")
            nc.vector.tensor_tensor(gall[:], gsum[:], g2[:], op=Alu.add)
            pgen_col = ph1.tile([128, 1], f32, tag="pgen")
            nc.scalar.activation(
                pgen_col[:], gall[:], Act.Sigmoid, bias=wxb[:], scale=1.0
            )
            nc.sync.dma_start(pgen_dram[rows, :], pgen_col[:])

        def _phase1b():
            pg_flat = pgen_dram[:, 0]
            pg_bcast = bass.AP(
                tensor=pg_flat.tensor, offset=pg_flat.offset,
                ap=[[0, 128]] + list(pg_flat.ap),
            )
            nc.gpsimd.dma_start(pgen_all[:], pg_bcast)
            nc.vector.tensor_scalar(
                om_all[:], pgen_all[:], -1.0, 1.0, Alu.mult, Alu.add
            )
            for blk in range(RB):
                for c in range(SP):
                    nc.vector.tensor_tensor(
                        scaledT[:, blk * SP + c, :],
                        attnT[:, blk * SP + c, :],
                        om_all[:, blk * 128:(blk + 1) * 128],
                        op=Alu.mult,
                    )

        vocab_v = vocab_d[:].rearrange("r (p f) -> p r f", p=128)
        out_v = out_d[:].rearrange("r (p f) -> p r f", p=128)

        def _phase2():
          for grp in range(NG):
            gr = slice(grp * G, (grp + 1) * G)
            vt = vpool.tile([128, G, FP], f32)
            nc.gpsimd.memset(vt[:, :, FD:FP], 0.0)
            nc.sync.dma_start(vt[:, :, 0:FD], vocab_v[:, gr, :])
            ot = opool.tile([128, G, FD], odt)
            for j in range(G):
                r = grp * G + j
                blk = r // 128
                rl = r % 128
                pg_sc = pgen_all[:, r:r + 1]
                if j % 2 == 0:
                    psb = ppool.tile([128, 2, FP], f32)
                ps = psb[:, j % 2, :]
                D = dqpool.tile([128, 128], f32, tag="D")
                if d_eng == "pool":
                    nc.gpsimd.tensor_scalar(
                        D[:], ident[:], pg_sc, None, Alu.mult
                    )
                else:
                    nc.scalar.mul(D[:], ident[:], pg_sc)
                nc.tensor.matmul(
                    ps, lhsT=D[:].bitcast(f32r), rhs=vt[:, j, :].bitcast(f32r),
                    start=True, stop=False,
                )
                for c in range(SP):
                    ch = blk * SP + c
                    A = abpool.tile([128, 128], bf16, tag="A")
                    a_e = nc.gpsimd if c < a_pool else nc.vector
                    a_e.tensor_scalar(
                        A[:], iotaP[:], piT[:, ch, rl:rl + 1],
                        scaledT[:, ch, rl:rl + 1], Alu.is_equal, Alu.mult,
                    )
                    Bt = abpool.tile([128, FD], bf16, tag="B")
                    nc.vector.tensor_scalar(
                        Bt[:], iotaF[:], fiT[:, ch, rl:rl + 1], None,
                        Alu.is_equal,
                    )
                    nc.tensor.matmul(
                        ps[:, 0:FD], lhsT=A[:], rhs=Bt[:],
                        start=False, stop=(c == SP - 1),
                    )
                if j % 2 == 1:
                    nc.scalar.copy(ot[:, j - 1:j + 1, :], psb[:, :, 0:FD])
            nc.scalar.dma_start(out_v[:, gr, :], ot[:])

        for _ in range(rep):
            _phase1a()
            _phase1b()
            _phase2()

    nc.compile()
    _PROGRAM_CACHE[key] = nc
    return nc


K_LS = 16   # local_scatter bucket capacity per (row, partition)


def build_program_v5(R=R_FULL, FD=FD_FULL, G=8, K=K_LS, LC=1, out_dt="f32",
                     base="f32", ablate="full", rep=1):
    """v5: scatter via GPSIMD local_scatter.

    Host pre-buckets each row's 400 (p=v//250, f=v%250, val) triples by
    target partition p (pure integer/data-movement). Device per row:
      - DVE scales the bucketed raw attn values by (1-p_gen)  [128,K] bf16
      - Pool local_scatter places them: scat[p, f] = val       [128,250] bf16
      - PE: ps = (pgen*I) @ vocab_row  (fp32 diag, starts PSUM group)
            ps += I_bf16 @ scat        (identity merge)
            ps += A_l @ B_l            (one leftover one-hot chunk: intra-row
                                        duplicate (p,f) pairs + bucket
                                        overflow; usually ~2 entries)
      - ACT copies PSUM->SBUF (pairs), group store from SBUF.
    """
    key = ("v5", R, FD, G, K, LC, out_dt, base, ablate, rep)
    if key in _PROGRAM_CACHE:
        return _PROGRAM_CACHE[key]

    from contextlib import ExitStack

    import concourse.bass as bass
    import concourse.tile as tile
    from concourse import bacc, mybir

    f32 = mybir.dt.float32
    bf16 = mybir.dt.bfloat16
    i16 = mybir.dt.int16
    Alu = mybir.AluOpType
    Act = mybir.ActivationFunctionType
    VL = 128 * FD
    RB = R // 128
    NG = R // G
    assert R % 128 == 0 and R % G == 0 and G % 2 == 0

    nc = bacc.Bacc("TRN2", target_bir_lowering=False, debug=False)

    vdt = f32 if base == "f32" else bf16
    ctx_d = nc.dram_tensor("ctx", [R, H], f32, kind="ExternalInput")
    hid_d = nc.dram_tensor("hid", [R, H], f32, kind="ExternalInput")
    trg_d = nc.dram_tensor("trg", [R, H], f32, kind="ExternalInput")
    # vocab/out live transposed in DRAM ([p, r, f], host pre/post transpose)
    # so each group DMA is one large contiguous descriptor per partition.
    vocab_d = nc.dram_tensor("vocab", [128, R, FD], vdt, kind="ExternalInput")
    dls_d = nc.dram_tensor("dls", [128, R, K], bf16, kind="ExternalInput")
    ils_d = nc.dram_tensor("ils", [128, R, K], i16, kind="ExternalInput")
    attnL_d = nc.dram_tensor("attnL", [128, LC * RB, 128], f32, kind="ExternalInput")
    piL_d = nc.dram_tensor("piL", [128, LC * RB, 128], f32, kind="ExternalInput")
    fiL_d = nc.dram_tensor("fiL", [128, LC * RB, 128], f32, kind="ExternalInput")
    wh_d = nc.dram_tensor("wh", [128, H], f32, kind="ExternalInput")
    ws_d = nc.dram_tensor("ws", [128, H], f32, kind="ExternalInput")
    wx_d = nc.dram_tensor("wx", [128, H], f32, kind="ExternalInput")
    wxb_d = nc.dram_tensor("wxb", [128, 1], f32, kind="ExternalInput")
    iotaP_d = nc.dram_tensor("iotaP", [128, 128], bf16, kind="ExternalInput")
    iotaF_d = nc.dram_tensor("iotaF", [128, FD], bf16, kind="ExternalInput")
    ident_d = nc.dram_tensor("ident", [128, 128], f32, kind="ExternalInput")
    identb_d = nc.dram_tensor("identb", [128, 128], bf16, kind="ExternalInput")
    odt = f32 if out_dt == "f32" else bf16
    out_d = nc.dram_tensor("out", [128, R, FD], odt, kind="ExternalOutput")

    with tile.TileContext(nc) as tc, ExitStack() as es:
        singles = es.enter_context(tc.tile_pool(name="singles", bufs=1))
        ph1 = es.enter_context(tc.tile_pool(name="ph1", bufs=2))
        vpool = es.enter_context(tc.tile_pool(name="vpool", bufs=6))
        opool = es.enter_context(tc.tile_pool(name="opool", bufs=3))
        abpool = es.enter_context(tc.tile_pool(name="ab", bufs=6))
        scpool = es.enter_context(tc.tile_pool(name="sc", bufs=6))
        ppool = es.enter_context(tc.tile_pool(name="psum", bufs=8, space="PSUM"))
        dpool = es.enter_context(tc.tile_pool(name="dram", bufs=1, space="DRAM"))

        dls = singles.tile([128, R, K], bf16)
        nc.sync.dma_start(dls[:], dls_d[:])
        ils = singles.tile([128, R, K], i16)
        nc.sync.dma_start(ils[:], ils_d[:])
        attnL = singles.tile([128, LC * RB, 128], f32)
        nc.sync.dma_start(attnL[:], attnL_d[:])
        piL = singles.tile([128, LC * RB, 128], f32)
        nc.sync.dma_start(piL[:], piL_d[:])
        fiL = singles.tile([128, LC * RB, 128], f32)
        nc.sync.dma_start(fiL[:], fiL_d[:])
        iotaP = singles.tile([128, 128], bf16)
        nc.sync.dma_start(iotaP[:], iotaP_d[:])
        iotaF = singles.tile([128, FD], bf16)
        nc.sync.dma_start(iotaF[:], iotaF_d[:])
        ident = singles.tile([128, 128], f32)
        nc.sync.dma_start(ident[:], ident_d[:])
        identb = singles.tile([128, 128], bf16)
        nc.sync.dma_start(identb[:], identb_d[:])
        wh = singles.tile([128, H], f32)
        nc.sync.dma_start(wh[:], wh_d[:])
        ws = singles.tile([128, H], f32)
        nc.sync.dma_start(ws[:], ws_d[:])
        wx = singles.tile([128, H], f32)
        nc.sync.dma_start(wx[:], wx_d[:])
        wxb = singles.tile([128, 1], f32)
        nc.sync.dma_start(wxb[:], wxb_d[:])
        scaledL = singles.tile([128, LC * RB, 128], f32)
        pgen_all = singles.tile([128, R], f32)
        om_all = singles.tile([128, R], f32)
        pgen_dram = dpool.tile([R, 1], f32)

        def _phase1a():
          for blk in range(RB):
            rows = slice(blk * 128, (blk + 1) * 128)
            gacc = ph1.tile([128, 1], f32, tag="gacc")
            gtmp = ph1.tile([128, 1], f32, tag="gtmp")
            g2 = ph1.tile([128, 1], f32, tag="g2")
            prod = ph1.tile([128, H], f32, tag="prod")
            for i, (src_d, w) in enumerate(
                ((ctx_d, wh), (hid_d, ws), (trg_d, wx))
            ):
                x = ph1.tile([128, H], f32, tag="x")
                nc.sync.dma_start(x[:], src_d[rows, :])
                nc.vector.tensor_tensor(prod[:], x[:], w[:], op=Alu.mult)
                dst = (gacc, gtmp, g2)[i]
                nc.vector.tensor_reduce(
                    dst[:], prod[:], axis=mybir.AxisListType.X, op=Alu.add
                )
            gsum = ph1.tile([128, 1], f32, tag="gsum")
            nc.vector.tensor_tensor(gsum[:], gacc[:], gtmp[:], op=Alu.add)
            gall = ph1.tile([128, 1], f32, tag="gall")
            nc.vector.tensor_tensor(gall[:], gsum[:], g2[:], op=Alu.add)
            pgen_col = ph1.tile([128, 1], f32, tag="pgen")
            nc.scalar.activation(
                pgen_col[:], gall[:], Act.Sigmoid, bias=wxb[:], scale=1.0
            )
            nc.sync.dma_start(pgen_dram[rows, :], pgen_col[:])

        def _phase1b():
            pg_flat = pgen_dram[:, 0]
            pg_bcast = bass.AP(
                tensor=pg_flat.tensor, offset=pg_flat.offset,
                ap=[[0, 128]] + list(pg_flat.ap),
            )
            nc.gpsimd.dma_start(pgen_all[:], pg_bcast)
            nc.vector.tensor_scalar(
                om_all[:], pgen_all[:], -1.0, 1.0, Alu.mult, Alu.add
            )
            for lb in range(LC * RB):
                blk = lb % RB
                nc.vector.tensor_tensor(
                    scaledL[:, lb, :],
                    attnL[:, lb, :],
                    om_all[:, blk * 128:(blk + 1) * 128],
                    op=Alu.mult,
                )

        vocab_v = vocab_d[:]
        out_v = out_d[:]

        vt_pre = {}

        def _preload(grp):
            gr = slice(grp * G, (grp + 1) * G)
            vt = vpool.tile([128, G, FD], vdt)
            nc.sync.dma_start(vt[:], vocab_v[:, gr, :])
            vt_pre[grp] = vt

        def _phase2():
          for grp in range(NG):
            gr = slice(grp * G, (grp + 1) * G)
            if grp in vt_pre:
                vt = vt_pre.pop(grp)
            else:
                vt = vpool.tile([128, G, FD], vdt)
                nc.sync.dma_start(vt[:], vocab_v[:, gr, :])
            ot = opool.tile([128, G, FD], odt)
            if ablate == "dmaonly":
                nc.scalar.copy(ot[:], vt[:])
                nc.scalar.dma_start(out_v[:, gr, :], ot[:])
                continue
            for j in range(G):
                r = grp * G + j
                blk = r // 128
                rl = r % 128
                pg_sc = pgen_all[:, r:r + 1]
                om_sc = om_all[:, r:r + 1]
                if j % 2 == 0:
                    psb = ppool.tile([128, 2, FD], f32)
                ps = psb[:, j % 2, :]
                # base: (pgen * I) @ vocab_row, starts the PSUM accum group
                if base == "f32":
                    D = abpool.tile([128, 128], f32, tag="D")
                    nc.vector.tensor_scalar(D[:], ident[:], pg_sc, None,
                                            Alu.mult)
                else:
                    D = abpool.tile([128, 128], bf16, tag="D")
                    nc.vector.tensor_scalar(D[:], identb[:], pg_sc, None,
                                            Alu.mult)
                nc.tensor.matmul(ps, lhsT=D[:], rhs=vt[:, j, :],
                                 start=True, stop=False)
                # scatter: scale bucketed values, local_scatter, identity-merge
                if ablate != "noscat":
                    sval = scpool.tile([128, K], bf16, tag="sval")
                    nc.vector.tensor_scalar(sval[:], dls[:, r, :], om_sc, None,
                                            Alu.mult)
                    scat = scpool.tile([128, FD], bf16, tag="scat")
                    nc.gpsimd.local_scatter(
                        scat[:], sval[:], ils[:, r, :],
                        channels=128, num_elems=FD, num_idxs=K,
                    )
                    nc.tensor.matmul(ps, lhsT=identb[:], rhs=scat[:],
                                     start=False, stop=False)
                # leftover chunks (duplicates/overflow): one-hot matmuls
                for l in range(LC if ablate != "noleft" else 0):
                    lb = l * RB + blk
                    A = abpool.tile([128, 128], bf16, tag="A")
                    nc.vector.tensor_scalar(
                        A[:], iotaP[:], piL[:, lb, rl:rl + 1],
                        scaledL[:, lb, rl:rl + 1], Alu.is_equal, Alu.mult,
                    )
                    Bt = abpool.tile([128, FD], bf16, tag="B")
                    nc.vector.tensor_scalar(
                        Bt[:], iotaF[:], fiL[:, lb, rl:rl + 1], None,
                        Alu.is_equal,
                    )
                    nc.tensor.matmul(ps, lhsT=A[:], rhs=Bt[:],
                                     start=False, stop=(l == LC - 1))
                if ablate == "noleft":
                    # close the accum group on a zero-contribution matmul
                    nc.tensor.matmul(ps, lhsT=identb[:], rhs=scat[:],
                                     start=False, stop=True)
                if j % 2 == 1:
                    nc.scalar.copy(ot[:, j - 1:j + 1, :], psb[:, :, :])
            nc.scalar.dma_start(out_v[:, gr, :], ot[:])

        for _ in range(rep):
            for g in range(4):
                _preload(g)
            _phase1a()
            _phase1b()
            _phase2()

    nc.compile()
    _PROGRAM_CACHE[key] = nc
    return nc


def make_core_inputs(ctx, hid, trg, vocab, attn, ids, w_h, w_s, w_x_w, w_x_b,
                     R=R_FULL, FD=FD_FULL, SP=SP_FULL):
    """Host-side prep for one core: flatten rows, decompose + transpose indices.

    ctx/hid/trg: [R, H] f32; vocab: [R, 128*FD] f32; attn: [R, S'] f32;
    ids: [R, S'] int. Returns the in_map dict for this core.
    """
    RB = R // 128
    Sp = SP * 128
    Sl = attn.shape[1]
    f32 = np.float32

    ids = np.asarray(ids).astype(np.int64)
    pi = (ids // FD).astype(f32)
    fi = (ids % FD).astype(f32)

    def tr(x, pad):
        full = np.full((R, Sp), pad, dtype=f32)
        full[:, :Sl] = x
        # [R, Sp] -> [RB, 128(r), SP, 128(s)] -> [s, RB, SP, r]
        t = full.reshape(RB, 128, SP, 128).transpose(3, 0, 2, 1)
        return np.ascontiguousarray(t.reshape(128, RB * SP, 128))

    def rep(w, n):
        return np.ascontiguousarray(
            np.broadcast_to(np.asarray(w, dtype=f32).reshape(1, n), (128, n))
        )

    return {
        "ctx": np.ascontiguousarray(ctx, dtype=f32),
        "hid": np.ascontiguousarray(hid, dtype=f32),
        "trg": np.ascontiguousarray(trg, dtype=f32),
        "vocab": np.ascontiguousarray(vocab, dtype=f32),
        "attnT": tr(np.asarray(attn, dtype=f32), 0.0),
        "piT": tr(pi, 1.0e4),
        "fiT": tr(fi, -1.0),
        "wh": rep(w_h, H),
        "ws": rep(w_s, H),
        "wx": rep(w_x_w, H),
        "wxb": rep(w_x_b, 1),
        "iotaP": rep(np.arange(128, dtype=f32), 128).astype(bfloat16),
        "iotaF": rep(np.arange(FD, dtype=f32), FD).astype(bfloat16),
        "ident": np.eye(128, dtype=f32),
    }


def make_core_inputs_v5(ctx, hid, trg, vocab, attn, ids, w_h, w_s, w_x_w,
                        w_x_b, R=R_FULL, FD=FD_FULL, K=K_LS):
    """Host prep for one core, v5 layout: bucket each row's (p=v//FD,
    f=v%FD, val) triples by target partition p. Integer index work plus
    value placement only — all arithmetic on the values happens on device.

    Layer 1 (local_scatter): first occurrence of each (row, p, f), up to K
    per (row, p). Everything else (duplicate (p,f) pairs, bucket overflow)
    goes to one leftover one-hot chunk per row (capacity 128).
    """
    RB = R // 128
    Sl = ids.shape[1]
    f32 = np.float32

    ids = np.asarray(ids).astype(np.int64)
    attn = np.asarray(attn, dtype=f32)
    NS = R * Sl
    rr = np.repeat(np.arange(R), Sl)
    pp = (ids // FD).ravel()
    ff = (ids % FD).ravel()
    vv = attn.ravel()

    order = np.lexsort((ff, pp, rr))
    rs, ps, fs, vs = rr[order], pp[order], ff[order], vv[order]
    idx = np.arange(NS)
    new_rpf = np.r_[True, (rs[1:] != rs[:-1]) | (ps[1:] != ps[:-1])
                    | (fs[1:] != fs[:-1])]
    new_rp = np.r_[True, (rs[1:] != rs[:-1]) | (ps[1:] != ps[:-1])]
    new_r = np.r_[True, rs[1:] != rs[:-1]]
    keep = new_rpf
    kc0 = np.cumsum(keep) - keep          # kept strictly before element
    rp_start = np.maximum.accumulate(np.where(new_rp, idx, -1))
    rank = kc0 - kc0[rp_start]            # rank among kept within (r, p)
    layer1 = keep & (rank < K)

    data_ls = np.zeros((128, R, K), dtype=bfloat16)
    idx_ls = np.full((128, R, K), -1, dtype=np.int16)
    m = layer1
    data_ls[ps[m], rs[m], rank[m]] = vs[m].astype(bfloat16)
    idx_ls[ps[m], rs[m], rank[m]] = fs[m].astype(np.int16)

    lm = ~layer1
    lc0 = np.cumsum(lm) - lm
    r_start = np.maximum.accumulate(np.where(new_r, idx, -1))
    lslot = (lc0 - lc0[r_start])[lm]
    LC = 1 if lslot.size == 0 else int(lslot.max()) // 128 + 1
    lr = rs[lm]
    attnL = np.zeros((128, LC, RB, 128), dtype=f32)
    piL = np.full((128, LC, RB, 128), 1.0e4, dtype=f32)
    fiL = np.full((128, LC, RB, 128), -1.0, dtype=f32)
    attnL[lslot % 128, lslot // 128, lr // 128, lr % 128] = vs[lm]
    piL[lslot % 128, lslot // 128, lr // 128, lr % 128] = ps[lm].astype(f32)
    fiL[lslot % 128, lslot // 128, lr // 128, lr % 128] = fs[lm].astype(f32)
    attnL = attnL.reshape(128, LC * RB, 128)
    piL = piL.reshape(128, LC * RB, 128)
    fiL = fiL.reshape(128, LC * RB, 128)

    def rep(w, n):
        return np.ascontiguousarray(
            np.broadcast_to(np.asarray(w, dtype=f32).reshape(1, n), (128, n))
        )

    return {
        "ctx": np.ascontiguousarray(ctx, dtype=f32),
        "hid": np.ascontiguousarray(hid, dtype=f32),
        "trg": np.ascontiguousarray(trg, dtype=f32),
        "vocab": np.ascontiguousarray(
            np.asarray(vocab).reshape(R, 128, FD).transpose(1, 0, 2).astype(
                f32 if V5_KW.get("base", "f32") == "f32" else bfloat16)),
        "dls": data_ls,
        "ils": idx_ls,
        "attnL": attnL,
        "piL": piL,
        "fiL": fiL,
        "wh": rep(w_h, H),
        "ws": rep(w_s, H),
        "wx": rep(w_x_w, H),
        "wxb": rep(w_x_b, 1),
        "iotaP": rep(np.arange(128, dtype=f32), 128).astype(bfloat16),
        "iotaF": rep(np.arange(FD, dtype=f32), FD).astype(bfloat16),
        "ident": np.eye(128, dtype=f32),
        "identb": np.eye(128, dtype=f32).astype(bfloat16),
    }


def make_in_maps(context_vecs, hidden, trg_embs, vocab_dists, attn_dists,
                 src_ids, w_h, w_s, w_x_w, w_x_b):
    """Build the 8 per-core input dicts from full inputs."""
    context_vecs = np.asarray(context_vecs)
    hidden = np.asarray(hidden)
    trg_embs = np.asarray(trg_embs)
    vocab_dists = np.asarray(vocab_dists)
    attn_dists = np.asarray(attn_dists)
    src_ids = np.asarray(src_ids)

    mk = make_core_inputs_v5 if VARIANT == "v5" else make_core_inputs
    in_maps = []
    for i in range(N_CORES):
        bs = slice(i * BPC, (i + 1) * BPC)
        in_maps.append(mk(
            context_vecs[bs].reshape(R_FULL, H),
            hidden[bs].reshape(R_FULL, H),
            trg_embs[bs].reshape(R_FULL, H),
            vocab_dists[bs].reshape(R_FULL, V),
            attn_dists[bs].reshape(R_FULL, S),
            src_ids[bs].reshape(R_FULL, S),
            w_h, w_s, w_x_w, w_x_b,
        ))
    if VARIANT == "v5":
        # all cores must share one program: pad leftover chunks to max LC
        RB = R_FULL // 128
        lc_max = max(m["piL"].shape[1] // RB for m in in_maps)
        for m in in_maps:
            lc = m["piL"].shape[1] // RB
            if lc < lc_max:
                pad = ((0, 0), (0, (lc_max - lc) * RB), (0, 0))
                m["attnL"] = np.pad(m["attnL"], pad)
                m["piL"] = np.pad(m["piL"], pad, constant_values=1.0e4)
                m["fiL"] = np.pad(m["fiL"], pad, constant_values=-1.0)
        global _LAST_LC
        _LAST_LC = lc_max
    return in_maps


VARIANT = "v5"          # "v5", "v4", or "diag" (previous baseline)
V4_KW = dict(G=8, a_pool=1, d_eng="pool", out_dt="f32")
V5_KW = dict(G=8, out_dt="f32", base="bf16")
_LAST_LC = 1            # leftover-chunk count of the last make_in_maps


def build_current(rep=1):
    if VARIANT == "v5":
        return build_program_v5(rep=rep, LC=_LAST_LC, **V5_KW)
    if VARIANT == "v4":
        return build_program_v4(rep=rep, **V4_KW)
    return build_program(rep=rep)


def kernel(context_vecs, hidden, trg_embs, vocab_dists, attn_dists,
           src_ids, pad_id, w_h, w_s, w_x_w, w_x_b):
    """Full-input entry point. Shards over 8 NeuronCores, returns [B,T,V] f32."""
    from concourse.bass_utils import run_bass_kernel_spmd

    in_maps = make_in_maps(context_vecs, hidden, trg_embs, vocab_dists,
                           attn_dists, src_ids, w_h, w_s, w_x_w, w_x_b)
    nc = build_current()
    res = run_bass_kernel_spmd(nc, in_maps, list(range(N_CORES)))
    outs = []
    for i in range(N_CORES):
        o = np.asarray(res.results[i]["out"]).astype(np.float32)
        if VARIANT == "v5":
            # device layout [128, R, FD] -> [R, 128*FD]
            o = o.transpose(1, 0, 2).reshape(R_FULL, V)
        outs.append(o.reshape(BPC, T, V))
    return np.concatenate(outs, axis=0)

